# revision 12
# baseline (speedup 1.0000x reference)
"""Chord sparse-attention module — Bass/Tile kernel for 8 TRN2 NeuronCores.

Problem (hardcoded): B=2, N=4096, E=256, H=512, N_W=12 mixing layers,
L=13 chord links at offsets [0,1,2,4,...,2048] (mod N, per batch).

    V = gelu(V @ gW1 + gb1) @ gW2 + gb2                      # g-MLP
    for k in 0..11:
        W_k = gelu(data @ fsW1[k] + fsb1[k]) @ fsW2[k] + fsb2[k]   # [B,N,13]
        V[n] = sum_l W_k[n,l] * V[(n+off_l) % N] + V[n]

Sharding: 8 cores = (batch b in {0,1}) x (row-quarter q in {0..3}).
Core (b,q) owns rows [q*1024, (q+1)*1024) of batch b:
  - computes its own g-MLP rows and all 12 of its own W_k row-slices
    (PE matmuls, bf16) -- no weight communication at all;
  - chord mixing per layer: per-partition-scalar muls on DVE/ACT/Pool
    (tensor_scalar, 4x bf16) + identity-matmul accumulation into PSUM
    (fp32) on the TensorEngine;
  - the only cross-core traffic is a per-layer AllGather of the updated
    V slices (bf16) within each 4-core batch group, since chord offsets
    reach 2048 rows ahead (= 2 neighbor slices).

Outputs: each core writes its own [1024, 256] f32 slice; the host
assembles the full [2, 4096, 256] output.
"""

import os

os.environ.setdefault("JAX_PLATFORMS", "cpu,axon")

import numpy as np
import ml_dtypes

B, N, E, H = 2, 4096, 256, 512
NW = 12
L = 13
OFFS = [0] + [1 << k for k in range(L - 1)]  # 0,1,2,...,2048
ROWS = N // 4          # 1024 rows per core
RT = ROWS // 128       # 8 row-tiles per core
LT = 24                # local Vg tiles: own 8 + two neighbor slices
N_CORES = 8

_BF16 = ml_dtypes.bfloat16

# link -> engine assignment for the per-partition-scalar muls (by link idx;
# OFFS[li]): li 1..7 read the DMA-shifted copies, li 0/8..12 read vin直接.
# li 12 is fused as the final DVE scalar_tensor_tensor (w*V + psum -> SBUF),
# replacing the ACT mul + PE add + psum->sbuf copy for that link.
ACT_LINKS = {10, 11}   # offs 512, 1024
POOL_LINKS = {0, 8, 9}  # offs 0, 128, 256



def _build_program():
    import concourse.bass as bass
    import concourse.bacc as bacc
    import concourse.tile as tile
    import concourse.mybir as mybir

    F32 = mybir.dt.float32
    BF16 = mybir.dt.bfloat16
    U32 = mybir.dt.uint32
    AF = mybir.ActivationFunctionType
    ALU = mybir.AluOpType

    nc = bacc.Bacc(
        "TRN2", target_bir_lowering=False, debug=False,
        enable_asserts=True, num_devices=N_CORES,
    )

    # ---- DRAM I/O ----------------------------------------------------
    vT_d = nc.dram_tensor("vT", [128, 2, ROWS], BF16, kind="ExternalInput")
    dT_d = nc.dram_tensor("dT", [128, 2, ROWS], BF16, kind="ExternalInput")
    gw1_d = nc.dram_tensor("gw1", [128, 2, H], BF16, kind="ExternalInput")
    gw2_d = nc.dram_tensor("gw2", [128, 4, E], BF16, kind="ExternalInput")
    gb1_d = nc.dram_tensor("gb1", [128, 4], F32, kind="ExternalInput")
    gb2_d = nc.dram_tensor("gb2", [1, E], BF16, kind="ExternalInput")
    fw1_d = nc.dram_tensor("fw1", [128, 2, NW, H], BF16, kind="ExternalInput")
    fw2_d = nc.dram_tensor("fw2", [128, 4, NW, L], BF16, kind="ExternalInput")
    fb1_d = nc.dram_tensor("fb1", [128, NW, 4], F32, kind="ExternalInput")
    fb2_d = nc.dram_tensor("fb2", [1, NW, L], BF16, kind="ExternalInput")
    eye_d = nc.dram_tensor("eye", [128, 128], BF16, kind="ExternalInput")
    off_d = nc.dram_tensor("offs", [1, 2], U32, kind="ExternalInput")
    out_d = nc.dram_tensor("out", [ROWS, E], F32, kind="ExternalOutput")

    with tile.TileContext(nc) as tc:
        with (
            tc.tile_pool(name="persist", bufs=1) as P,
            tc.tile_pool(name="hidp", bufs=2) as HP,
            tc.tile_pool(name="tmp", bufs=16) as TP,
            tc.tile_pool(name="psH", bufs=2, space="PSUM") as PSH,
            tc.tile_pool(name="psA", bufs=4, space="PSUM") as PSA,
            tc.tile_pool(name="psW", bufs=2, space="PSUM") as PSW,
            tc.tile_pool(name="dram", bufs=2, space="DRAM") as DP,
        ):
            # ---- persistent SBUF tiles + input DMAs ------------------
            def load(dram_t, shape, dt):
                t = P.tile(shape, dt, tag=f"ld_{dram_t.name}")
                nc.sync.dma_start(t[:], dram_t.ap())
                return t

            vT = load(vT_d, [128, 2, ROWS], BF16)
            dT = load(dT_d, [128, 2, ROWS], BF16)
            gw1 = load(gw1_d, [128, 2, H], BF16)
            gw2 = load(gw2_d, [128, 4, E], BF16)
            gb1 = load(gb1_d, [128, 4], F32)
            gb2 = load(gb2_d, [1, E], BF16)
            fw1 = load(fw1_d, [128, 2, NW, H], BF16)
            fw2 = load(fw2_d, [128, 4, NW, L], BF16)
            fb1 = load(fb1_d, [128, NW, 4], F32)
            fb2 = load(fb2_d, [1, NW, L], BF16)
            eye = load(eye_d, [128, 128], BF16)

            ones = P.tile([1, 128], BF16, tag="ones")
            nc.vector.memset(ones[:], 1.0)

            W_sb = P.tile([128, NW, RT, L], F32, tag="W_sb")
            vga = P.tile([128, LT, E], BF16, tag="vga")
            vgb = P.tile([128, LT, E], BF16, tag="vgb")
            vbufs = [vga, vgb]
            stage = P.tile([128, RT, E], F32, tag="stage")
            # partition-shifted copies of V (engines can't read partition-
            # offset APs; DMA builds row-shifted views instead)
            SH_RS = [1, 2, 4, 8, 16, 32, 64]
            vsh = {}
            for r in SH_RS:
                vsh_r = P.tile([128, RT, E], BF16, tag=f"vsh{r}", name=f"vsh{r}")
                vsh[r] = vsh_r

            def make_shifts(vg):
                """vsh[r][p, t, :] = V row (t*128 + p + r), t in 0..7."""
                for i, r in enumerate(SH_RS):
                    eng = [nc.sync, nc.scalar, nc.gpsimd][i % 3]
                    eng.dma_start(
                        vsh[r][0 : 128 - r, 0:RT, :], vg[r:128, 0:RT, :]
                    )
                    eng.dma_start(
                        vsh[r][128 - r : 128, 0 : RT - 1, :],
                        vg[0:r, 1:RT, :],
                    )
                    eng.dma_start(
                        vsh[r][128 - r : 128, RT - 1 : RT, :],
                        vg[0:r, RT : RT + 1, :],
                    )

            # dynamic row offsets into the gathered slab (per-core data)
            r_off = []
            for i in range(2):
                reg = nc.sync.alloc_register(f"agoff{i}")
                nc.sync.reg_load(reg, off_d[0:1, i : i + 1])
                r_off.append(
                    nc.sync.snap(reg, donate=True, min_val=0, max_val=N - ROWS)
                )

            # ---- shared MLP pieces -----------------------------------
            def mlp_hidden(w1, bias_ap_fn, k_tag, gate=None):
                """hid^T = gelu(x^T-GEMM): returns [128, 4, ROWS] bf16 tile."""
                src = dT if k_tag >= 0 else vT
                hid = HP.tile([128, 4, ROWS], BF16, tag="hid")
                for half in range(2):
                    rs = slice(half * 512, half * 512 + 512)
                    for hc in range(4):
                        ph = PSH.tile([128, 512], F32)
                        for ec in range(2):
                            mm = nc.tensor.matmul(
                                ph[:],
                                lhsT=w1[:, ec, hc * 128 : (hc + 1) * 128]
                                if k_tag < 0
                                else w1[:, ec, k_tag, hc * 128 : (hc + 1) * 128],
                                rhs=src[:, ec, rs],
                                start=(ec == 0),
                                stop=(ec == 1),
                            )
                            if gate is not None and ec == 0:
                                tile.add_dep_helper(
                                    mm.ins, gate.ins, sync=False,
                                    reason="stagger wgen into its AG window",
                                )
                        nc.scalar.activation(
                            hid[:, hc, rs], ph[:], AF.Gelu,
                            bias=bias_ap_fn(hc),
                        )
                return hid

            def wgen(k, gate=None):
                hid = mlp_hidden(fw1, lambda hc: fb1[:, k, hc : hc + 1], k, gate)
                for t in range(RT):
                    pw = PSW.tile([128, L], F32)
                    for hc in range(4):
                        nc.tensor.matmul(
                            pw[:],
                            lhsT=hid[:, hc, t * 128 : (t + 1) * 128],
                            rhs=fw2[:, hc, k, :],
                            start=(hc == 0),
                            stop=False,
                        )
                    nc.tensor.matmul(
                        pw[:], lhsT=ones[0:1, 0:128], rhs=fb2[0:1, k, :],
                        start=False, stop=True,
                    )
                    nc.vector.tensor_copy(W_sb[:, k, t, :], pw[:])

            def bounce_and_gather(vg):
                """own tiles 0..7 of vg -> DRAM -> AllGather -> tiles 8..24."""
                bounce = DP.tile([ROWS, E], BF16, tag="bounce")
                bview = bounce[:].rearrange("(t p) e -> p t e", p=128)
                for j in range(0, RT, 2):
                    nc.sync.dma_start(bview[:, j : j + 2, :], vg[:, j : j + 2, :])
                gath = DP.tile([N, E], BF16, tag="gath")
                nc.gpsimd.collective_compute(
                    "AllGather",
                    ALU.bypass,
                    replica_groups=[[0, 1, 2, 3], [4, 5, 6, 7]],
                    ins=[bounce[:].opt()],
                    outs=[gath[:].opt()],
                )
                for i in range(2):
                    src = gath[bass.ds(r_off[i], ROWS), :].rearrange(
                        "(t p) e -> p t e", p=128
                    )
                    for j in range(0, RT, 4):
                        nc.sync.dma_start(
                            vg[:, 8 + i * 8 + j : 8 + i * 8 + j + 4, :],
                            src[:, j : j + 4, :],
                        )

            # ---- g-MLP -> V_0 ----------------------------------------
            ghid = mlp_hidden(gw1, lambda hc: gb1[:, hc : hc + 1], -1)
            for t in range(RT):
                pa = PSA.tile([128, E], F32)
                for hc in range(4):
                    nc.tensor.matmul(
                        pa[:],
                        lhsT=ghid[:, hc, t * 128 : (t + 1) * 128],
                        rhs=gw2[:, hc, :],
                        start=(hc == 0),
                        stop=False,
                    )
                nc.tensor.matmul(
                    pa[:], lhsT=ones[0:1, 0:128], rhs=gb2[0:1, :],
                    start=False, stop=True,
                )
                nc.vector.tensor_copy(vga[:, t, :], pa[:])
            bounce_and_gather(vga)
            make_shifts(vga)

            # ---- 12 chord-mixing layers ------------------------------
            mix_marker = [None] * NW
            for k in range(NW):
                wgen(k, gate=mix_marker[k - 1] if k >= 1 else None)
                vin = vbufs[k % 2]
                vout = vbufs[(k + 1) % 2]
                for t in range(RT):
                    pa = PSA.tile([128, E], F32)
                    for li, off in enumerate(OFFS):
                        qk, r = divmod(off, 128)
                        w_col = W_sb[:, k, t, li : li + 1]
                        srcv = vin[:, t + qk, :] if r == 0 else vsh[r][:, t, :]
                        if li == L - 1:
                            dst = vout[:, t, :] if k < NW - 1 else stage[:, t, :]
                            stt = nc.vector.scalar_tensor_tensor(
                                dst, srcv, w_col, pa[:],
                                op0=ALU.mult, op1=ALU.add,
                            )
                            if t == 0:
                                mix_marker[k] = stt
                            continue
                        tmp = TP.tile([128, E], BF16, tag="tmp")
                        if li in ACT_LINKS:
                            nc.scalar.activation(
                                tmp[:], srcv, AF.Copy, scale=w_col,
                            )
                        elif li in POOL_LINKS:
                            nc.gpsimd.tensor_scalar_mul(tmp[:], srcv, w_col)
                        else:
                            nc.vector.tensor_scalar_mul(tmp[:], srcv, w_col)
                        nc.tensor.matmul(
                            pa[:], lhsT=eye[:], rhs=tmp[:],
                            start=(li == 0), stop=(li == L - 2),
                        )
                if k < NW - 1:
                    bounce_and_gather(vout)
                    make_shifts(vout)

            # ---- write output ----------------------------------------
            oview = out_d.ap().rearrange("(t p) e -> p t e", p=128)
            for j in range(0, RT, 2):
                nc.sync.dma_start(oview[:, j : j + 2, :], stage[:, j : j + 2, :])

    nc.compile()
    return nc


_NC_CACHE = None


def _get_program():
    global _NC_CACHE
    if _NC_CACHE is None:
        _NC_CACHE = _build_program()
    return _NC_CACHE


def _make_in_maps(V, data, gW1, gb1, gW2, gb2, fsW1, fsb1, fsW2, fsb2):
    f32 = np.float32
    bf = _BF16

    def bfT_rows(x):  # [1024, 256] -> [128, 2, 1024] (x^T in e-chunks)
        xT = np.ascontiguousarray(x.astype(bf).T)          # [256, 1024]
        return np.ascontiguousarray(xT.reshape(2, 128, ROWS).transpose(1, 0, 2))

    gw1_h = np.ascontiguousarray(
        gW1.astype(bf).reshape(2, 128, H).transpose(1, 0, 2))      # [128,2,H]
    gw2_h = np.ascontiguousarray(
        gW2.astype(bf).reshape(4, 128, E).transpose(1, 0, 2))      # [128,4,E]
    gb1_h = np.ascontiguousarray(gb1.astype(f32).reshape(4, 128).T)  # [128,4]
    gb2_h = gb2.astype(bf).reshape(1, E)
    fw1_h = np.ascontiguousarray(
        fsW1.astype(bf).reshape(NW, 2, 128, H).transpose(2, 1, 0, 3))  # [128,2,NW,H]
    fw2_h = np.ascontiguousarray(
        fsW2.astype(bf).reshape(NW, 4, 128, L).transpose(2, 1, 0, 3))  # [128,4,NW,L]
    fb1_h = np.ascontiguousarray(
        fsb1.astype(f32).reshape(NW, 4, 128).transpose(2, 0, 1))   # [128,NW,4]
    fb2_h = fsb2.astype(f32).copy()
    fb2_h[:, 0] += 1.0                    # fold the residual into link 0
    fb2_h = fb2_h.astype(bf).reshape(1, NW, L)
    eye_h = np.eye(128, dtype=f32).astype(bf)

    in_maps = []
    for c in range(N_CORES):
        b, q = divmod(c, 4)
        r0 = q * ROWS
        in_maps.append({
            "vT": bfT_rows(V[b, r0 : r0 + ROWS]),
            "dT": bfT_rows(data[b, r0 : r0 + ROWS]),
            "gw1": gw1_h, "gw2": gw2_h, "gb1": gb1_h, "gb2": gb2_h,
            "fw1": fw1_h, "fw2": fw2_h, "fb1": fb1_h, "fb2": fb2_h,
            "eye": eye_h,
            "offs": np.array(
                [[((q + 1) % 4) * ROWS, ((q + 2) % 4) * ROWS]], np.uint32
            ),
        })
    return in_maps


_LAST_RESULTS = None  # stashed for test.py introspection (exec time etc.)


def kernel(**inputs) -> np.ndarray:
    from concourse import bass_utils

    nc = _get_program()
    in_maps = _make_in_maps(
        np.asarray(inputs["V"], np.float32),
        np.asarray(inputs["data"], np.float32),
        np.asarray(inputs["gW1"], np.float32),
        np.asarray(inputs["gb1"], np.float32),
        np.asarray(inputs["gW2"], np.float32),
        np.asarray(inputs["gb2"], np.float32),
        np.asarray(inputs["fsW1"], np.float32),
        np.asarray(inputs["fsb1"], np.float32),
        np.asarray(inputs["fsW2"], np.float32),
        np.asarray(inputs["fsb2"], np.float32),
    )
    res = bass_utils.run_bass_kernel_spmd(
        nc, in_maps, core_ids=list(range(N_CORES)),
        trace=bool(int(os.environ.get("KERNEL_TRACE", "0"))),
    )
    global _LAST_RESULTS
    _LAST_RESULTS = res

    out = np.empty((B, N, E), np.float32)
    for c in range(N_CORES):
        b, q = divmod(c, 4)
        out[b, q * ROWS : (q + 1) * ROWS, :] = res.results[c]["out"]
    return out


if __name__ == "__main__":
    rng = np.random.default_rng(0)
    ins = {
        "V": rng.standard_normal((B, N, E)).astype(np.float32),
        "data": rng.standard_normal((B, N, E)).astype(np.float32),
        "gW1": rng.standard_normal((E, H)).astype(np.float32) * 0.02,
        "gb1": np.zeros((H,), np.float32),
        "gW2": rng.standard_normal((H, E)).astype(np.float32) * 0.02,
        "gb2": np.zeros((E,), np.float32),
        "fsW1": rng.standard_normal((NW, E, H)).astype(np.float32) * 0.02,
        "fsb1": np.zeros((NW, H), np.float32),
        "fsW2": rng.standard_normal((NW, H, L)).astype(np.float32) * 0.02,
        "fsb2": np.zeros((NW, L), np.float32),
        "cols": ((np.arange(N)[:, None] + np.array(OFFS)[None, :]) % N).astype(np.int32),
    }
    out = kernel(**ins)
    print(out.shape, out.dtype)


# revision 14
# speedup vs baseline: 1.0003x; 1.0003x over previous
"""Chord sparse-attention module — Bass/Tile kernel for 8 TRN2 NeuronCores.

Problem (hardcoded): B=2, N=4096, E=256, H=512, N_W=12 mixing layers,
L=13 chord links at offsets [0,1,2,4,...,2048] (mod N, per batch).

    V = gelu(V @ gW1 + gb1) @ gW2 + gb2                      # g-MLP
    for k in 0..11:
        W_k = gelu(data @ fsW1[k] + fsb1[k]) @ fsW2[k] + fsb2[k]   # [B,N,13]
        V[n] = sum_l W_k[n,l] * V[(n+off_l) % N] + V[n]

Sharding: 8 cores = (batch b in {0,1}) x (row-quarter q in {0..3}).
Core (b,q) owns rows [q*1024, (q+1)*1024) of batch b:
  - computes its own g-MLP rows and all 12 of its own W_k row-slices
    (PE matmuls, bf16) -- no weight communication at all;
  - chord mixing per layer: per-partition-scalar muls on DVE/ACT/Pool
    (tensor_scalar, 4x bf16) + identity-matmul accumulation into PSUM
    (fp32) on the TensorEngine;
  - the only cross-core traffic is a per-layer AllGather of the updated
    V slices (bf16) within each 4-core batch group, since chord offsets
    reach 2048 rows ahead (= 2 neighbor slices).

Outputs: each core writes its own [1024, 256] f32 slice; the host
assembles the full [2, 4096, 256] output.
"""

import os

os.environ.setdefault("JAX_PLATFORMS", "cpu,axon")

import numpy as np
import ml_dtypes

B, N, E, H = 2, 4096, 256, 512
NW = 12
L = 13
OFFS = [0] + [1 << k for k in range(L - 1)]  # 0,1,2,...,2048
ROWS = N // 4          # 1024 rows per core
RT = ROWS // 128       # 8 row-tiles per core
LT = 24                # local Vg tiles: own 8 + two neighbor slices
N_CORES = 8

_BF16 = ml_dtypes.bfloat16

# link -> engine assignment for the per-partition-scalar muls (by link idx;
# OFFS[li]): li 1..7 read the DMA-shifted copies, li 0/8..12 read vin直接.
# li 12 is fused as the final DVE scalar_tensor_tensor (w*V + psum -> SBUF),
# replacing the ACT mul + PE add + psum->sbuf copy for that link.
ACT_LINKS = {10, 11}   # offs 512, 1024
POOL_LINKS = {0, 8, 9}  # offs 0, 128, 256



def _build_program():
    import concourse.bass as bass
    import concourse.bacc as bacc
    import concourse.tile as tile
    import concourse.mybir as mybir

    F32 = mybir.dt.float32
    BF16 = mybir.dt.bfloat16
    U32 = mybir.dt.uint32
    AF = mybir.ActivationFunctionType
    ALU = mybir.AluOpType

    nc = bacc.Bacc(
        "TRN2", target_bir_lowering=False, debug=False,
        enable_asserts=True, num_devices=N_CORES,
    )

    # ---- DRAM I/O ----------------------------------------------------
    vT_d = nc.dram_tensor("vT", [128, 2, ROWS], BF16, kind="ExternalInput")
    dT_d = nc.dram_tensor("dT", [128, 2, ROWS], BF16, kind="ExternalInput")
    gw1_d = nc.dram_tensor("gw1", [128, 2, H], BF16, kind="ExternalInput")
    gw2_d = nc.dram_tensor("gw2", [128, 4, E], BF16, kind="ExternalInput")
    gb1_d = nc.dram_tensor("gb1", [128, 4], F32, kind="ExternalInput")
    gb2_d = nc.dram_tensor("gb2", [1, E], BF16, kind="ExternalInput")
    fw1_d = nc.dram_tensor("fw1", [128, 2, NW, H], BF16, kind="ExternalInput")
    fw2_d = nc.dram_tensor("fw2", [128, 4, NW, L], BF16, kind="ExternalInput")
    fb1_d = nc.dram_tensor("fb1", [128, NW, 4], F32, kind="ExternalInput")
    fb2_d = nc.dram_tensor("fb2", [1, NW, L], BF16, kind="ExternalInput")
    eye_d = nc.dram_tensor("eye", [128, 128], BF16, kind="ExternalInput")
    off_d = nc.dram_tensor("offs", [1, 2], U32, kind="ExternalInput")
    out_d = nc.dram_tensor("out", [ROWS, E], F32, kind="ExternalOutput")

    with tile.TileContext(nc) as tc:
        with (
            tc.tile_pool(name="persist", bufs=1) as P,
            tc.tile_pool(name="hidp", bufs=2) as HP,
            tc.tile_pool(name="tmp", bufs=24) as TP,
            tc.tile_pool(name="psH", bufs=2, space="PSUM") as PSH,
            tc.tile_pool(name="psA", bufs=4, space="PSUM") as PSA,
            tc.tile_pool(name="psW", bufs=2, space="PSUM") as PSW,
            tc.tile_pool(name="dram", bufs=2, space="DRAM") as DP,
        ):
            # ---- persistent SBUF tiles + input DMAs ------------------
            def load(dram_t, shape, dt):
                t = P.tile(shape, dt, tag=f"ld_{dram_t.name}")
                nc.sync.dma_start(t[:], dram_t.ap())
                return t

            vT = load(vT_d, [128, 2, ROWS], BF16)
            dT = load(dT_d, [128, 2, ROWS], BF16)
            gw1 = load(gw1_d, [128, 2, H], BF16)
            gw2 = load(gw2_d, [128, 4, E], BF16)
            gb1 = load(gb1_d, [128, 4], F32)
            gb2 = load(gb2_d, [1, E], BF16)
            fw1 = load(fw1_d, [128, 2, NW, H], BF16)
            fw2 = load(fw2_d, [128, 4, NW, L], BF16)
            fb1 = load(fb1_d, [128, NW, 4], F32)
            fb2 = load(fb2_d, [1, NW, L], BF16)
            eye = load(eye_d, [128, 128], BF16)

            ones = P.tile([1, 128], BF16, tag="ones")
            nc.vector.memset(ones[:], 1.0)

            W_sb = P.tile([128, NW, RT, L], F32, tag="W_sb")
            vga = P.tile([128, LT, E], BF16, tag="vga")
            vgb = P.tile([128, LT, E], BF16, tag="vgb")
            vbufs = [vga, vgb]
            stage = P.tile([128, RT, E], F32, tag="stage")
            # partition-shifted copies of V (engines can't read partition-
            # offset APs; DMA builds row-shifted views instead)
            SH_RS = [1, 2, 4, 8, 16, 32, 64]
            vsh = {}
            for r in SH_RS:
                vsh_r = P.tile([128, RT, E], BF16, tag=f"vsh{r}", name=f"vsh{r}")
                vsh[r] = vsh_r

            def make_shifts(vg):
                """vsh[r][p, t, :] = V row (t*128 + p + r), t in 0..7."""
                for i, r in enumerate(SH_RS):
                    eng = [nc.sync, nc.scalar, nc.gpsimd][i % 3]
                    eng.dma_start(
                        vsh[r][0 : 128 - r, 0:RT, :], vg[r:128, 0:RT, :]
                    )
                    eng.dma_start(
                        vsh[r][128 - r : 128, 0 : RT - 1, :],
                        vg[0:r, 1:RT, :],
                    )
                    eng.dma_start(
                        vsh[r][128 - r : 128, RT - 1 : RT, :],
                        vg[0:r, RT : RT + 1, :],
                    )

            # dynamic row offsets into the gathered slab (per-core data)
            r_off = []
            for i in range(2):
                reg = nc.sync.alloc_register(f"agoff{i}")
                nc.sync.reg_load(reg, off_d[0:1, i : i + 1])
                r_off.append(
                    nc.sync.snap(reg, donate=True, min_val=0, max_val=N - ROWS)
                )

            # ---- shared MLP pieces -----------------------------------
            def mlp_hidden(w1, bias_ap_fn, k_tag, gate=None):
                """hid^T = gelu(x^T-GEMM): returns [128, 4, ROWS] bf16 tile."""
                src = dT if k_tag >= 0 else vT
                hid = HP.tile([128, 4, ROWS], BF16, tag="hid")
                for half in range(2):
                    rs = slice(half * 512, half * 512 + 512)
                    for hc in range(4):
                        ph = PSH.tile([128, 512], F32)
                        for ec in range(2):
                            mm = nc.tensor.matmul(
                                ph[:],
                                lhsT=w1[:, ec, hc * 128 : (hc + 1) * 128]
                                if k_tag < 0
                                else w1[:, ec, k_tag, hc * 128 : (hc + 1) * 128],
                                rhs=src[:, ec, rs],
                                start=(ec == 0),
                                stop=(ec == 1),
                            )
                            if gate is not None and ec == 0:
                                tile.add_dep_helper(
                                    mm.ins, gate.ins, sync=False,
                                    reason="stagger wgen into its AG window",
                                )
                        nc.scalar.activation(
                            hid[:, hc, rs], ph[:], AF.Gelu,
                            bias=bias_ap_fn(hc),
                        )
                return hid

            def wgen(k, gate=None):
                hid = mlp_hidden(fw1, lambda hc: fb1[:, k, hc : hc + 1], k, gate)
                for t in range(RT):
                    pw = PSW.tile([128, L], F32)
                    for hc in range(4):
                        nc.tensor.matmul(
                            pw[:],
                            lhsT=hid[:, hc, t * 128 : (t + 1) * 128],
                            rhs=fw2[:, hc, k, :],
                            start=(hc == 0),
                            stop=False,
                        )
                    nc.tensor.matmul(
                        pw[:], lhsT=ones[0:1, 0:128], rhs=fb2[0:1, k, :],
                        start=False, stop=True,
                    )
                    nc.vector.tensor_copy(W_sb[:, k, t, :], pw[:])

            def bounce_and_gather(vg):
                """own tiles 0..7 of vg -> DRAM -> AllGather -> tiles 8..24."""
                bounce = DP.tile([ROWS, E], BF16, tag="bounce")
                bview = bounce[:].rearrange("(t p) e -> p t e", p=128)
                for j in range(0, RT, 2):
                    nc.sync.dma_start(bview[:, j : j + 2, :], vg[:, j : j + 2, :])
                gath = DP.tile([N, E], BF16, tag="gath")
                nc.gpsimd.collective_compute(
                    "AllGather",
                    ALU.bypass,
                    replica_groups=[[0, 1, 2, 3], [4, 5, 6, 7]],
                    ins=[bounce[:].opt()],
                    outs=[gath[:].opt()],
                )
                for i in range(2):
                    src = gath[bass.ds(r_off[i], ROWS), :].rearrange(
                        "(t p) e -> p t e", p=128
                    )
                    for j in range(0, RT, 2):
                        nc.sync.dma_start(
                            vg[:, 8 + i * 8 + j : 8 + i * 8 + j + 2, :],
                            src[:, j : j + 2, :],
                        )

            # ---- g-MLP -> V_0 ----------------------------------------
            ghid = mlp_hidden(gw1, lambda hc: gb1[:, hc : hc + 1], -1)
            for t in range(RT):
                pa = PSA.tile([128, E], F32)
                for hc in range(4):
                    nc.tensor.matmul(
                        pa[:],
                        lhsT=ghid[:, hc, t * 128 : (t + 1) * 128],
                        rhs=gw2[:, hc, :],
                        start=(hc == 0),
                        stop=False,
                    )
                nc.tensor.matmul(
                    pa[:], lhsT=ones[0:1, 0:128], rhs=gb2[0:1, :],
                    start=False, stop=True,
                )
                nc.vector.tensor_copy(vga[:, t, :], pa[:])
            bounce_and_gather(vga)
            make_shifts(vga)

            # ---- 12 chord-mixing layers ------------------------------
            mix_marker = [None] * NW
            for k in range(NW):
                wgen(k, gate=mix_marker[k - 1] if k >= 1 else None)
                vin = vbufs[k % 2]
                vout = vbufs[(k + 1) % 2]
                for t in range(RT):
                    pa = PSA.tile([128, E], F32)
                    for li, off in enumerate(OFFS):
                        qk, r = divmod(off, 128)
                        w_col = W_sb[:, k, t, li : li + 1]
                        srcv = vin[:, t + qk, :] if r == 0 else vsh[r][:, t, :]
                        if li == L - 1:
                            dst = vout[:, t, :] if k < NW - 1 else stage[:, t, :]
                            stt = nc.vector.scalar_tensor_tensor(
                                dst, srcv, w_col, pa[:],
                                op0=ALU.mult, op1=ALU.add,
                            )
                            if t == 0:
                                mix_marker[k] = stt
                            continue
                        tmp = TP.tile([128, E], BF16, tag="tmp")
                        if li in ACT_LINKS:
                            nc.scalar.activation(
                                tmp[:], srcv, AF.Copy, scale=w_col,
                            )
                        elif li in POOL_LINKS:
                            nc.gpsimd.tensor_scalar_mul(tmp[:], srcv, w_col)
                        else:
                            nc.vector.tensor_scalar_mul(tmp[:], srcv, w_col)
                        nc.tensor.matmul(
                            pa[:], lhsT=eye[:], rhs=tmp[:],
                            start=(li == 0), stop=(li == L - 2),
                        )
                if k < NW - 1:
                    bounce_and_gather(vout)
                    make_shifts(vout)

            # ---- write output ----------------------------------------
            oview = out_d.ap().rearrange("(t p) e -> p t e", p=128)
            for j in range(0, RT, 2):
                nc.sync.dma_start(oview[:, j : j + 2, :], stage[:, j : j + 2, :])

    nc.compile()
    return nc


_NC_CACHE = None


def _get_program():
    global _NC_CACHE
    if _NC_CACHE is None:
        _NC_CACHE = _build_program()
    return _NC_CACHE


def _make_in_maps(V, data, gW1, gb1, gW2, gb2, fsW1, fsb1, fsW2, fsb2):
    f32 = np.float32
    bf = _BF16

    def bfT_rows(x):  # [1024, 256] -> [128, 2, 1024] (x^T in e-chunks)
        xT = np.ascontiguousarray(x.astype(bf).T)          # [256, 1024]
        return np.ascontiguousarray(xT.reshape(2, 128, ROWS).transpose(1, 0, 2))

    gw1_h = np.ascontiguousarray(
        gW1.astype(bf).reshape(2, 128, H).transpose(1, 0, 2))      # [128,2,H]
    gw2_h = np.ascontiguousarray(
        gW2.astype(bf).reshape(4, 128, E).transpose(1, 0, 2))      # [128,4,E]
    gb1_h = np.ascontiguousarray(gb1.astype(f32).reshape(4, 128).T)  # [128,4]
    gb2_h = gb2.astype(bf).reshape(1, E)
    fw1_h = np.ascontiguousarray(
        fsW1.astype(bf).reshape(NW, 2, 128, H).transpose(2, 1, 0, 3))  # [128,2,NW,H]
    fw2_h = np.ascontiguousarray(
        fsW2.astype(bf).reshape(NW, 4, 128, L).transpose(2, 1, 0, 3))  # [128,4,NW,L]
    fb1_h = np.ascontiguousarray(
        fsb1.astype(f32).reshape(NW, 4, 128).transpose(2, 0, 1))   # [128,NW,4]
    fb2_h = fsb2.astype(f32).copy()
    fb2_h[:, 0] += 1.0                    # fold the residual into link 0
    fb2_h = fb2_h.astype(bf).reshape(1, NW, L)
    eye_h = np.eye(128, dtype=f32).astype(bf)

    in_maps = []
    for c in range(N_CORES):
        b, q = divmod(c, 4)
        r0 = q * ROWS
        in_maps.append({
            "vT": bfT_rows(V[b, r0 : r0 + ROWS]),
            "dT": bfT_rows(data[b, r0 : r0 + ROWS]),
            "gw1": gw1_h, "gw2": gw2_h, "gb1": gb1_h, "gb2": gb2_h,
            "fw1": fw1_h, "fw2": fw2_h, "fb1": fb1_h, "fb2": fb2_h,
            "eye": eye_h,
            "offs": np.array(
                [[((q + 1) % 4) * ROWS, ((q + 2) % 4) * ROWS]], np.uint32
            ),
        })
    return in_maps


_LAST_RESULTS = None  # stashed for test.py introspection (exec time etc.)


def kernel(**inputs) -> np.ndarray:
    from concourse import bass_utils

    nc = _get_program()
    in_maps = _make_in_maps(
        np.asarray(inputs["V"], np.float32),
        np.asarray(inputs["data"], np.float32),
        np.asarray(inputs["gW1"], np.float32),
        np.asarray(inputs["gb1"], np.float32),
        np.asarray(inputs["gW2"], np.float32),
        np.asarray(inputs["gb2"], np.float32),
        np.asarray(inputs["fsW1"], np.float32),
        np.asarray(inputs["fsb1"], np.float32),
        np.asarray(inputs["fsW2"], np.float32),
        np.asarray(inputs["fsb2"], np.float32),
    )
    res = bass_utils.run_bass_kernel_spmd(
        nc, in_maps, core_ids=list(range(N_CORES)),
        trace=bool(int(os.environ.get("KERNEL_TRACE", "0"))),
    )
    global _LAST_RESULTS
    _LAST_RESULTS = res

    out = np.empty((B, N, E), np.float32)
    for c in range(N_CORES):
        b, q = divmod(c, 4)
        out[b, q * ROWS : (q + 1) * ROWS, :] = res.results[c]["out"]
    return out


if __name__ == "__main__":
    rng = np.random.default_rng(0)
    ins = {
        "V": rng.standard_normal((B, N, E)).astype(np.float32),
        "data": rng.standard_normal((B, N, E)).astype(np.float32),
        "gW1": rng.standard_normal((E, H)).astype(np.float32) * 0.02,
        "gb1": np.zeros((H,), np.float32),
        "gW2": rng.standard_normal((H, E)).astype(np.float32) * 0.02,
        "gb2": np.zeros((E,), np.float32),
        "fsW1": rng.standard_normal((NW, E, H)).astype(np.float32) * 0.02,
        "fsb1": np.zeros((NW, H), np.float32),
        "fsW2": rng.standard_normal((NW, H, L)).astype(np.float32) * 0.02,
        "fsb2": np.zeros((NW, L), np.float32),
        "cols": ((np.arange(N)[:, None] + np.array(OFFS)[None, :]) % N).astype(np.int32),
    }
    out = kernel(**ins)
    print(out.shape, out.dtype)
